# revision 13
# baseline (speedup 1.0000x reference)
"""Single-head attention (B=4, T=4096, C=1024, H=64) on 8 trn2 NeuronCores.

Sharding: 8 shards = (batch b, query-half h).  Each core receives x[b]
pre-transposed to xT [C=1024, T=4096] in fp16; for h==1 the T columns are
rotated by 2048 so "this core's" 2048 queries are always columns 0:2048
(softmax is permutation-invariant over keys).  SPMD: identical program on
every core, no rank logic.

v2 (all-fp16 + dual-engine exp + host-side normalize):
  phase 1: stream xT in fp16 [128,512] tiles.
    blocks 0..3 (query half):  PE pass_q  [Wq*0.125 | Wv]^T x -> rows 0:64 =
    Q^T (scaled), rows 64:128 = V^T;  PE pass_k  Wk^T x -> rows 0:64 = K^T.
    blocks 4..7:               PE pass_kv [Wk | Wv]^T x  (K^T low, V^T high).
    GPSIMD copies PSUM->SBUF fp16: qt_sb[64,2048] (partitions 0:64),
    kvt_sb[128,4096] (K^T rows 0:64, V^T rows 64:128).  PE transposes V^T
    tiles (stationary from partitions 64:128) -> va [128, kt, 65] fp16, with
    a ones column at index 64 (denominator comes out of the attnV matmul).
  phase 2: per 1024-query chunk (2 of them), per key tile [128]:
    PE: scoresT [128,1024] = kvt[0:64, tile].T @ qt  (K=64 contraction, the
        1/8 softmax scale is folded into Wq host-side)
    exp, alternating by tile parity:
      ACT: ex = Exp(scoresT)                       -> fp16 SBUF
      DVE: Schraudolph bit-trick exp: ex_bits = int16(scoresT*A + B); the
        int16 bit pattern IS fp16(exp(s)) to ~1.5% rms (denominator uses the
        same approximated values, so most of the error divides out).
    PE: acc[65,512] += va[:,kt,:].T @ ex  (accumulate over 32 tiles; row 64
        of acc = sum of exp = softmax denominator)
  epilogue: GPSIMD copies acc -> fp16, DMA out [65,2048] = numerator rows
  0:64 + denominator row 64.  Host divides + transposes (the "unshard").
"""

import os
import sys

for _p in ("/opt/trn_rl_repo", "/root/.axon_site/_ro/trn_rl_repo"):
    if os.path.isdir(_p) and _p not in sys.path:
        sys.path.append(_p)

import numpy as np

import concourse.bacc as bacc
import concourse.mybir as mybir
import concourse.tile as tile
from concourse.bass_utils import run_bass_kernel_spmd
from concourse.masks import make_identity

B = 4
T = 4096
C = 1024
H = 64
TQ = T // 2  # queries per core
N_CORES = 8

F32 = mybir.dt.float32
F16 = mybir.dt.float16
I16 = mybir.dt.int16

NC_CH = C // 128  # 8 contraction chunks
NSB = T // 512  # 8 key/source blocks of 512
NST = T // 128  # 32 key tiles of 128
NTC = TQ // 512  # 4 query chunks of 512

# Schraudolph fp16 exp constants (fold the 1/8 score scale into Wq, so the
# DVE sees the already-scaled score): bits = s*1024/ln2 + (15*1024 - C).
# +0.5 turns the convert's truncation into round-half-up.  C=57 tuned by
# numpy simulation (rel-err plateau 55..65).
SCHRAUD_A = 1024.0 / float(np.log(2.0))
SCHRAUD_B = 15.0 * 1024.0 - 57.0 + 0.5


def _build_module():
    nc = bacc.Bacc("TRN2", target_bir_lowering=False, debug=False, num_devices=N_CORES)

    xT = nc.dram_tensor("xT", [NC_CH, NSB // 2, 128, 2, 512], F16, kind="ExternalInput").ap()
    # host-packed partition-major weight chunk tensors
    wqv = nc.dram_tensor("wqv", [128, NC_CH, 2 * H], F16, kind="ExternalInput").ap()
    wk = nc.dram_tensor("wk", [128, NC_CH, H], F16, kind="ExternalInput").ap()
    wkv = nc.dram_tensor("wkv", [128, NC_CH, 2 * H], F16, kind="ExternalInput").ap()
    out = nc.dram_tensor("out", [H + 1, TQ], F16, kind="ExternalOutput").ap()

    EXP = mybir.ActivationFunctionType.Exp
    MUL = mybir.AluOpType.mult
    ADD = mybir.AluOpType.add

    dma_engines = (nc.sync, nc.gpsimd, nc.scalar)

    with tile.TileContext(nc) as tc:
        with (
            tc.tile_pool(name="const", bufs=1) as const_pool,
            tc.tile_pool(name="xt", bufs=32) as xt_pool,
            tc.tile_pool(name="xt2", bufs=16) as xt2_pool,
            tc.tile_pool(name="big", bufs=1) as big_pool,
            tc.tile_pool(name="exa", bufs=6) as exa_pool,
            tc.tile_pool(name="exv", bufs=6) as exv_pool,
            tc.tile_pool(name="osb", bufs=2) as out_pool,
            tc.tile_pool(name="p1", bufs=2, space="PSUM") as psum_p1,
            tc.tile_pool(name="psc", bufs=4, space="PSUM") as psum_sc,
            tc.tile_pool(name="pacc", bufs=2, space="PSUM") as psum_acc,
        ):
            # ---- constants ----
            wqv_sb = const_pool.tile([128, NC_CH, 2 * H], F16, tag="wqv")
            wk_sb = const_pool.tile([128, NC_CH, H], F16, tag="wk")
            wkv_sb = const_pool.tile([128, NC_CH, 2 * H], F16, tag="wkv")
            ident = const_pool.tile([128, 128], F16, tag="ident")
            nc.sync.dma_start(wqv_sb[:, 0:4, :], wqv[:, 0:4, :])
            nc.gpsimd.dma_start(wqv_sb[:, 4:8, :], wqv[:, 4:8, :])
            nc.scalar.dma_start(wk_sb[:, 0:4, :], wk[:, 0:4, :])
            nc.sync.dma_start(wk_sb[:, 4:8, :], wk[:, 4:8, :])
            nc.gpsimd.dma_start(wkv_sb[:], wkv)

            # ---- persistent activations ----
            kvt_sb = big_pool.tile([128, T], F16, tag="kvt")  # K^T low | V^T high
            qt_sb = big_pool.tile([64, TQ], F16, tag="qt")  # Q^T (scaled)
            va = big_pool.tile([128, NST, H + 1], F16, tag="va")

            # ---- phase 1 ----
            def emit_dma_block(sb):
                xts = []
                for c in range(NC_CH):
                    xt = xt_pool.tile([128, 512], F16, tag="xt")
                    if sb < 1:
                        # split the first block across all queues for the
                        # earliest possible first matmul
                        dma_engines[c % 3].dma_start(
                            xt[:, 0:256], xT[c, sb // 2, :, sb % 2, 0:256]
                        )
                        dma_engines[(c + 1) % 3].dma_start(
                            xt[:, 256:512], xT[c, sb // 2, :, sb % 2, 256:512]
                        )
                    else:
                        # keep the scalar sequencer free for exp work
                        dma_engines[c % 2].dma_start(xt[:], xT[c, sb // 2, :, sb % 2, :])
                    xts.append(xt)
                return xts

            kv_pair_tiles = {}

            def emit_dma_pair(sb):  # one trigger per chunk covers sb, sb+1
                xts = []
                for c in range(NC_CH):
                    xt = xt2_pool.tile([128, 2, 512], F16, tag="xt2")
                    dma_engines[c % 2].dma_start(xt[:], xT[c, sb // 2])
                    xts.append(xt)
                kv_pair_tiles[sb] = xts

            def emit_transposes(sb):
                # V^T tiles of this block -> va [128, kt, 0:64]
                for j in range(4):
                    kt = sb * 4 + j
                    vt_ps = psum_p1.tile([128, H], F16, tag="p1")
                    nc.tensor.transpose(
                        vt_ps[:],
                        kvt_sb[64:128, kt * 128 : (kt + 1) * 128],
                        ident[64:128, 64:128],
                    )
                    nc.vector.tensor_copy(va[:, kt, 0:H], vt_ps[:])

            def emit_proj_q_block(sb, xts=None):  # blocks 0..3
                if xts is None:
                    xts = emit_dma_block(sb)
                col = slice(sb * 512, (sb + 1) * 512)
                qv_ps = psum_p1.tile([128, 512], F32, tag="p1")
                for c in range(NC_CH):
                    nc.tensor.matmul(
                        qv_ps[:],
                        wqv_sb[:, c, :],
                        xts[c][:],
                        start=(c == 0),
                        stop=(c == NC_CH - 1),
                    )
                nc.vector.tensor_copy(qt_sb[:, col], qv_ps[0:64, :])
                nc.vector.tensor_copy(kvt_sb[64:128, col], qv_ps[64:128, :])
                k_ps = psum_p1.tile([64, 512], F32, tag="p1")
                for c in range(NC_CH):
                    nc.tensor.matmul(
                        k_ps[:],
                        wk_sb[:, c, :],
                        xts[c][:],
                        start=(c == 0),
                        stop=(c == NC_CH - 1),
                    )
                nc.vector.tensor_copy(kvt_sb[0:64, col], k_ps[:])
                emit_transposes(sb)

            def emit_proj_kv_block(sb):  # blocks 4..7
                base = 4 + ((sb - 4) // 2) * 2
                xts = kv_pair_tiles[base]
                half = sb - base
                col = slice(sb * 512, (sb + 1) * 512)
                kv_ps = psum_p1.tile([128, 512], F32, tag="p1")
                for c in range(NC_CH):
                    nc.tensor.matmul(
                        kv_ps[:],
                        wkv_sb[:, c, :],
                        xts[c][:, half, :],
                        start=(c == 0),
                        stop=(c == NC_CH - 1),
                    )
                nc.vector.tensor_copy(kvt_sb[:, col], kv_ps[:])
                emit_transposes(sb)

            # ---- phase 2 (software-pipelined over half-chunk tiles) ----
            # tile t = (tcp, kt, i): scores [128,512] -> exp -> attnV.
            # The PE executes its queue in order, so scores are emitted
            # LOOKAHEAD tiles ahead of the matching attnV: while the exp
            # engines work on tile t, the PE computes scores t+1..t+LA.
            acc_tiles = {}
            ex_tiles = {}
            LOOKAHEAD = 4

            def get_acc(tcp):
                if tcp not in acc_tiles:
                    if tcp == 0:
                        oa = psum_acc.tile([H + 1, 512], F32, tag="acc", name="acc_a0")
                        ob = psum_acc.tile([H + 1, 512], F32, tag="acc", name="acc_b0")
                    else:
                        # tcp0's slots are free once emit_out(0) copies them
                        oa = psum_acc.tile([H + 1, 512], F32, tag="acc", name="acc_a1")
                        ob = psum_acc.tile([H + 1, 512], F32, tag="acc", name="acc_b1")
                    acc_tiles[tcp] = (oa, ob)
                return acc_tiles[tcp]

            def emit_scores_exp(t):
                tcp, kt, i = t
                if tcp == 1 and (kt * 2 + i) % 3 == 2:
                    sc = psum_p1.tile([128, 512], F32, tag="p1")
                else:
                    sc = psum_sc.tile([128, 512], F32, tag="sc")
                nc.tensor.matmul(
                    sc[:],
                    kvt_sb[0:64, kt * 128 : (kt + 1) * 128],
                    qt_sb[:, (2 * tcp + i) * 512 : (2 * tcp + i + 1) * 512],
                    start=True,
                    stop=True,
                )
                # ACT owns i==0, DVE owns i==1 (concurrent); in the early
                # window DVE is still doing phase-1 copies, so ACT picks up
                # some of its tiles.
                act_turn = (i == 0) ^ (kt % 2 == 1)
                if tcp == 0 and kt < 16 and kt % 3 == 2:
                    act_turn = True
                ex = (exa_pool if act_turn else exv_pool).tile(
                    [128, 512], F16, tag="ex"
                )
                if act_turn:
                    nc.scalar.activation(ex[:], sc[:], EXP)
                else:
                    nc.vector.tensor_scalar(
                        ex[:].bitcast(I16), sc[:], SCHRAUD_A, SCHRAUD_B, MUL, ADD
                    )
                ex_tiles[t] = ex

            def emit_attnv(t):
                tcp, kt, i = t
                acc = get_acc(tcp)[i]
                nc.tensor.matmul(
                    acc[:],
                    va[:, kt, :],
                    ex_tiles.pop(t)[:],
                    start=(kt == 0),
                    stop=(kt == NST - 1),
                )

            def emit_out_half(tcp, i):
                osb = out_pool.tile([H + 1, 512], F16, tag="osb")
                nc.vector.tensor_copy(osb[:], acc_tiles[tcp][i][:])
                nc.sync.dma_start(
                    out[:, tcp * 1024 + i * 512 : tcp * 1024 + (i + 1) * 512],
                    osb[:],
                )

            xts0 = emit_dma_block(0)
            warm_sb = big_pool.tile([128, 512], F16, tag="warm")
            nc.gpsimd.memset(warm_sb[:], 0.0)
            for _ in range(10):
                warm_ps = psum_sc.tile([128, 512], F32, tag="sc")
                nc.tensor.matmul(
                    warm_ps[:], warm_sb[:, 0:128], warm_sb[:], start=True, stop=True
                )
            make_identity(nc, ident[:])
            nc.gpsimd.memset(va[:, :, H : H + 1], 1.0)
            emit_proj_q_block(0, xts0)
            for sb in range(1, NTC):
                emit_proj_q_block(sb)
            emit_dma_pair(4)
            emit_dma_pair(6)

            tiles = [(tcp, kt, i) for tcp in (0, 1) for kt in range(NST)
                     for i in (0, 1)]
            for idx in range(len(tiles) + LOOKAHEAD):
                if idx >= LOOKAHEAD:
                    t = tiles[idx - LOOKAHEAD]
                    emit_attnv(t)
                    if t[1] == NST - 1:
                        emit_out_half(t[0], t[2])
                if idx < len(tiles):
                    tcp, kt, i = tiles[idx]
                    # stream the key-half projection blocks through the
                    # early attention tiles
                    if tcp == 0 and i == 0 and kt < 16 and kt % 4 == 0:
                        emit_proj_kv_block(NTC + kt // 4)
                    emit_scores_exp(tiles[idx])

    nc.compile()
    return nc


_NC_CACHE = None


def _get_module():
    global _NC_CACHE
    if _NC_CACHE is None:
        _NC_CACHE = _build_module()
    return _NC_CACHE


def _make_in_maps(x, Wq, Wk, Wv):
    f16 = np.float16
    xT = np.transpose(np.asarray(x, dtype=np.float32), (0, 2, 1))  # [B, C, T]
    wq = np.asarray(Wq, dtype=np.float32) * 0.125  # fold softmax scale
    wk_ = np.asarray(Wk, dtype=np.float32)
    wv = np.asarray(Wv, dtype=np.float32)

    def pack2(a, b):  # [C, H] x2 -> [128, NC_CH, 2H] partition-major fp16
        cat = np.concatenate([a, b], axis=1).reshape(NC_CH, 128, 2 * H)
        return np.ascontiguousarray(cat.transpose(1, 0, 2)).astype(f16)

    wqv = pack2(wq, wv)
    wkv = pack2(wk_, wv)
    wk1 = np.ascontiguousarray(
        wk_.reshape(NC_CH, 128, H).transpose(1, 0, 2)
    ).astype(f16)

    in_maps = []
    for core in range(N_CORES):
        b, h = divmod(core, 2)
        xt = xT[b]
        if h == 1:
            xt = np.concatenate([xt[:, TQ:], xt[:, :TQ]], axis=1)
        # [C, T] -> [NC_CH, NSB, 128, 512] fp16
        xt = np.ascontiguousarray(
            xt.reshape(NC_CH, 128, NSB // 2, 2, 512).transpose(0, 2, 1, 3, 4)
        ).astype(f16)
        in_maps.append({"xT": xt, "wqv": wqv, "wk": wk1, "wkv": wkv})
    return in_maps


def run(x, Wq, Wk, Wv, **spmd_kwargs):
    """Run on hardware; returns (output, BassKernelResults)."""
    nc = _get_module()
    in_maps = _make_in_maps(x, Wq, Wk, Wv)
    res = run_bass_kernel_spmd(nc, in_maps, core_ids=list(range(N_CORES)), **spmd_kwargs)
    out = np.empty((B, T, H), dtype=np.float32)
    for core in range(N_CORES):
        b, h = divmod(core, 2)
        o = np.asarray(res.results[core]["out"], dtype=np.float32)  # [65, TQ]
        out[b, h * TQ : (h + 1) * TQ, :] = (o[0:H, :] / o[H, :]).T
    return out, res


def kernel(x, Wq, Wk, Wv):
    out, _ = run(x, Wq, Wk, Wv)
    return out


# revision 14
# speedup vs baseline: 1.1328x; 1.1328x over previous
"""Single-head attention (B=4, T=4096, C=1024, H=64) on 8 trn2 NeuronCores.

Sharding: 8 shards = (batch b, query-half h).  Each core receives x[b]
pre-transposed to xT [C=1024, T=4096] in fp16; for h==1 the T columns are
rotated by 2048 so "this core's" 2048 queries are always columns 0:2048
(softmax is permutation-invariant over keys).  SPMD: identical program on
every core, no rank logic.

v2 (all-fp16 + dual-engine exp + host-side normalize):
  phase 1: stream xT in fp16 [128,512] tiles.
    blocks 0..3 (query half):  PE pass_q  [Wq*0.125 | Wv]^T x -> rows 0:64 =
    Q^T (scaled), rows 64:128 = V^T;  PE pass_k  Wk^T x -> rows 0:64 = K^T.
    blocks 4..7:               PE pass_kv [Wk | Wv]^T x  (K^T low, V^T high).
    GPSIMD copies PSUM->SBUF fp16: qt_sb[64,2048] (partitions 0:64),
    kvt_sb[128,4096] (K^T rows 0:64, V^T rows 64:128).  PE transposes V^T
    tiles (stationary from partitions 64:128) -> va [128, kt, 65] fp16, with
    a ones column at index 64 (denominator comes out of the attnV matmul).
  phase 2: per 1024-query chunk (2 of them), per key tile [128]:
    PE: scoresT [128,1024] = kvt[0:64, tile].T @ qt  (K=64 contraction, the
        1/8 softmax scale is folded into Wq host-side)
    exp, alternating by tile parity:
      ACT: ex = Exp(scoresT)                       -> fp16 SBUF
      DVE: Schraudolph bit-trick exp: ex_bits = int16(scoresT*A + B); the
        int16 bit pattern IS fp16(exp(s)) to ~1.5% rms (denominator uses the
        same approximated values, so most of the error divides out).
    PE: acc[65,512] += va[:,kt,:].T @ ex  (accumulate over 32 tiles; row 64
        of acc = sum of exp = softmax denominator)
  epilogue: GPSIMD copies acc -> fp16, DMA out [65,2048] = numerator rows
  0:64 + denominator row 64.  Host divides + transposes (the "unshard").
"""

import os
import sys

for _p in ("/opt/trn_rl_repo", "/root/.axon_site/_ro/trn_rl_repo"):
    if os.path.isdir(_p) and _p not in sys.path:
        sys.path.append(_p)

import numpy as np

import concourse.bacc as bacc
import concourse.mybir as mybir
import concourse.tile as tile
from concourse.bass_utils import run_bass_kernel_spmd
from concourse.masks import make_identity

B = 4
T = 4096
C = 1024
H = 64
TQ = T // 2  # queries per core
N_CORES = 8

F32 = mybir.dt.float32
F16 = mybir.dt.float16
I16 = mybir.dt.int16

NC_CH = C // 128  # 8 contraction chunks
NSB = T // 512  # 8 key/source blocks of 512
NST = T // 128  # 32 key tiles of 128
NTC = TQ // 512  # 4 query chunks of 512

# Schraudolph fp16 exp constants (fold the 1/8 score scale into Wq, so the
# DVE sees the already-scaled score): bits = s*1024/ln2 + (15*1024 - C).
# +0.5 turns the convert's truncation into round-half-up.  C=57 tuned by
# numpy simulation (rel-err plateau 55..65).
SCHRAUD_A = 1024.0 / float(np.log(2.0))
SCHRAUD_B = 15.0 * 1024.0 - 57.0 + 0.5


def _build_module():
    nc = bacc.Bacc("TRN2", target_bir_lowering=False, debug=False, num_devices=N_CORES)

    xT = nc.dram_tensor("xT", [NC_CH, NSB // 2, 128, 2, 512], F16, kind="ExternalInput").ap()
    # host-packed partition-major weight chunk tensors
    wqv = nc.dram_tensor("wqv", [128, NC_CH, 2 * H], F16, kind="ExternalInput").ap()
    wk = nc.dram_tensor("wk", [128, NC_CH, H], F16, kind="ExternalInput").ap()
    wkv = nc.dram_tensor("wkv", [128, NC_CH, 2 * H], F16, kind="ExternalInput").ap()
    out = nc.dram_tensor("out", [H + 1, TQ], F16, kind="ExternalOutput").ap()

    EXP = mybir.ActivationFunctionType.Exp
    MUL = mybir.AluOpType.mult
    ADD = mybir.AluOpType.add

    dma_engines = (nc.sync, nc.gpsimd, nc.scalar)

    with tile.TileContext(nc) as tc:
        with (
            tc.tile_pool(name="const", bufs=1) as const_pool,
            tc.tile_pool(name="xt", bufs=32) as xt_pool,
            tc.tile_pool(name="xt2", bufs=16) as xt2_pool,
            tc.tile_pool(name="big", bufs=1) as big_pool,
            tc.tile_pool(name="exa", bufs=6) as exa_pool,
            tc.tile_pool(name="exv", bufs=6) as exv_pool,
            tc.tile_pool(name="osb", bufs=2) as out_pool,
            tc.tile_pool(name="p1", bufs=2, space="PSUM") as psum_p1,
            tc.tile_pool(name="psc", bufs=4, space="PSUM") as psum_sc,
            tc.tile_pool(name="pacc", bufs=2, space="PSUM") as psum_acc,
        ):
            # ---- constants ----
            wqv_sb = const_pool.tile([128, NC_CH, 2 * H], F16, tag="wqv")
            wk_sb = const_pool.tile([128, NC_CH, H], F16, tag="wk")
            wkv_sb = const_pool.tile([128, NC_CH, 2 * H], F16, tag="wkv")
            ident = const_pool.tile([128, 128], F16, tag="ident")
            nc.sync.dma_start(wqv_sb[:, 0:4, :], wqv[:, 0:4, :])
            nc.gpsimd.dma_start(wqv_sb[:, 4:8, :], wqv[:, 4:8, :])
            nc.scalar.dma_start(wk_sb[:, 0:4, :], wk[:, 0:4, :])
            nc.sync.dma_start(wk_sb[:, 4:8, :], wk[:, 4:8, :])
            nc.gpsimd.dma_start(wkv_sb[:], wkv)

            # ---- persistent activations ----
            kvt_sb = big_pool.tile([128, T], F16, tag="kvt")  # K^T low | V^T high
            qt_sb = big_pool.tile([64, TQ], F16, tag="qt")  # Q^T (scaled)
            va = big_pool.tile([128, NST, H + 1], F16, tag="va")

            # ---- phase 1 ----
            def emit_dma_block(sb):
                xts = []
                for c in range(NC_CH):
                    xt = xt_pool.tile([128, 512], F16, tag="xt")
                    if sb < 1:
                        # split the first block across all queues for the
                        # earliest possible first matmul
                        dma_engines[c % 3].dma_start(
                            xt[:, 0:256], xT[c, sb // 2, :, sb % 2, 0:256]
                        )
                        dma_engines[(c + 1) % 3].dma_start(
                            xt[:, 256:512], xT[c, sb // 2, :, sb % 2, 256:512]
                        )
                    else:
                        # keep the scalar sequencer free for exp work
                        dma_engines[c % 2].dma_start(xt[:], xT[c, sb // 2, :, sb % 2, :])
                    xts.append(xt)
                return xts

            kv_pair_tiles = {}

            def emit_dma_pair(sb):  # one trigger per chunk covers sb, sb+1
                xts = []
                for c in range(NC_CH):
                    xt = xt2_pool.tile([128, 2, 512], F16, tag="xt2")
                    dma_engines[c % 2].dma_start(xt[:], xT[c, sb // 2])
                    xts.append(xt)
                kv_pair_tiles[sb] = xts

            def emit_transposes(sb):
                # V^T tiles of this block -> va [128, kt, 0:64]
                for j in range(4):
                    kt = sb * 4 + j
                    vt_ps = psum_p1.tile([128, H], F16, tag="p1")
                    nc.tensor.transpose(
                        vt_ps[:],
                        kvt_sb[64:128, kt * 128 : (kt + 1) * 128],
                        ident[64:128, 64:128],
                    )
                    nc.vector.tensor_copy(va[:, kt, 0:H], vt_ps[:])

            def emit_proj_q_block(sb, xts=None):  # blocks 0..3
                if xts is None:
                    xts = emit_dma_block(sb)
                col = slice(sb * 512, (sb + 1) * 512)
                qv_ps = psum_p1.tile([128, 512], F32, tag="p1")
                for c in range(NC_CH):
                    nc.tensor.matmul(
                        qv_ps[:],
                        wqv_sb[:, c, :],
                        xts[c][:],
                        start=(c == 0),
                        stop=(c == NC_CH - 1),
                    )
                nc.vector.tensor_copy(qt_sb[:, col], qv_ps[0:64, :])
                nc.vector.tensor_copy(kvt_sb[64:128, col], qv_ps[64:128, :])
                k_ps = psum_p1.tile([64, 512], F32, tag="p1")
                for c in range(NC_CH):
                    nc.tensor.matmul(
                        k_ps[:],
                        wk_sb[:, c, :],
                        xts[c][:],
                        start=(c == 0),
                        stop=(c == NC_CH - 1),
                    )
                nc.vector.tensor_copy(kvt_sb[0:64, col], k_ps[:])
                emit_transposes(sb)

            def emit_proj_kv_block(sb):  # blocks 4..7
                base = 4 + ((sb - 4) // 2) * 2
                xts = kv_pair_tiles[base]
                half = sb - base
                col = slice(sb * 512, (sb + 1) * 512)
                kv_ps = psum_p1.tile([128, 512], F32, tag="p1")
                for c in range(NC_CH):
                    nc.tensor.matmul(
                        kv_ps[:],
                        wkv_sb[:, c, :],
                        xts[c][:, half, :],
                        start=(c == 0),
                        stop=(c == NC_CH - 1),
                    )
                nc.vector.tensor_copy(kvt_sb[:, col], kv_ps[:])
                emit_transposes(sb)

            # ---- phase 2 (software-pipelined over half-chunk tiles) ----
            # tile t = (tcp, kt, i): scores [128,512] -> exp -> attnV.
            # The PE executes its queue in order, so scores are emitted
            # LOOKAHEAD tiles ahead of the matching attnV: while the exp
            # engines work on tile t, the PE computes scores t+1..t+LA.
            acc_tiles = {}
            ex_tiles = {}
            LOOKAHEAD = 6

            def get_acc(tcp):
                if tcp not in acc_tiles:
                    if tcp == 0:
                        oa = psum_acc.tile([H + 1, 512], F32, tag="acc", name="acc_a0")
                        ob = psum_acc.tile([H + 1, 512], F32, tag="acc", name="acc_b0")
                    else:
                        # projections are done; reuse the p1 psum slots
                        oa = psum_p1.tile([H + 1, 512], F32, tag="p1", name="acc_a1")
                        ob = psum_p1.tile([H + 1, 512], F32, tag="p1", name="acc_b1")
                    acc_tiles[tcp] = (oa, ob)
                return acc_tiles[tcp]

            def emit_scores_exp(t):
                tcp, kt, i = t
                sc = psum_sc.tile([128, 512], F32, tag="sc")
                nc.tensor.matmul(
                    sc[:],
                    kvt_sb[0:64, kt * 128 : (kt + 1) * 128],
                    qt_sb[:, (2 * tcp + i) * 512 : (2 * tcp + i + 1) * 512],
                    start=True,
                    stop=True,
                )
                # ACT owns i==0, DVE owns i==1 (concurrent); in the early
                # window DVE is still doing phase-1 copies, so ACT picks up
                # some of its tiles.
                act_turn = (i == 0) ^ (kt % 2 == 1)
                if tcp == 0 and kt < 16 and kt % 3 == 2:
                    act_turn = True
                ex = (exa_pool if act_turn else exv_pool).tile(
                    [128, 512], F16, tag="ex"
                )
                if act_turn:
                    nc.scalar.activation(ex[:], sc[:], EXP)
                else:
                    nc.vector.tensor_scalar(
                        ex[:].bitcast(I16), sc[:], SCHRAUD_A, SCHRAUD_B, MUL, ADD
                    )
                ex_tiles[t] = ex

            def emit_attnv(t):
                tcp, kt, i = t
                acc = get_acc(tcp)[i]
                nc.tensor.matmul(
                    acc[:],
                    va[:, kt, :],
                    ex_tiles.pop(t)[:],
                    start=(kt == 0),
                    stop=(kt == NST - 1),
                )

            def emit_out_half(tcp, i):
                osb = out_pool.tile([H + 1, 512], F16, tag="osb")
                nc.scalar.copy(osb[:], acc_tiles[tcp][i][:])
                nc.sync.dma_start(
                    out[:, tcp * 1024 + i * 512 : tcp * 1024 + (i + 1) * 512],
                    osb[:],
                )

            xts0 = emit_dma_block(0)
            warm_sb = big_pool.tile([128, 512], F16, tag="warm")
            nc.gpsimd.memset(warm_sb[:], 0.0)
            for _ in range(10):
                warm_ps = psum_sc.tile([128, 512], F32, tag="sc")
                nc.tensor.matmul(
                    warm_ps[:], warm_sb[:, 0:128], warm_sb[:], start=True, stop=True
                )
            make_identity(nc, ident[:])
            nc.gpsimd.memset(va[:, :, H : H + 1], 1.0)
            emit_proj_q_block(0, xts0)
            for sb in range(1, NTC):
                emit_proj_q_block(sb)
            emit_dma_pair(4)
            emit_dma_pair(6)

            tiles = [(tcp, kt, i) for tcp in (0, 1) for kt in range(NST)
                     for i in (0, 1)]
            for idx in range(len(tiles) + LOOKAHEAD):
                if idx < len(tiles):
                    tcp, kt, i = tiles[idx]
                    # stream the key-half projection blocks through the
                    # early attention tiles
                    if tcp == 0 and i == 0 and kt < 16 and kt % 4 == 0:
                        emit_proj_kv_block(NTC + kt // 4)
                    emit_scores_exp(tiles[idx])
                if idx >= LOOKAHEAD:
                    t = tiles[idx - LOOKAHEAD]
                    emit_attnv(t)
                    if t[1] == NST - 1:
                        emit_out_half(t[0], t[2])

    nc.compile()
    return nc


_NC_CACHE = None


def _get_module():
    global _NC_CACHE
    if _NC_CACHE is None:
        _NC_CACHE = _build_module()
    return _NC_CACHE


def _make_in_maps(x, Wq, Wk, Wv):
    f16 = np.float16
    xT = np.transpose(np.asarray(x, dtype=np.float32), (0, 2, 1))  # [B, C, T]
    wq = np.asarray(Wq, dtype=np.float32) * 0.125  # fold softmax scale
    wk_ = np.asarray(Wk, dtype=np.float32)
    wv = np.asarray(Wv, dtype=np.float32)

    def pack2(a, b):  # [C, H] x2 -> [128, NC_CH, 2H] partition-major fp16
        cat = np.concatenate([a, b], axis=1).reshape(NC_CH, 128, 2 * H)
        return np.ascontiguousarray(cat.transpose(1, 0, 2)).astype(f16)

    wqv = pack2(wq, wv)
    wkv = pack2(wk_, wv)
    wk1 = np.ascontiguousarray(
        wk_.reshape(NC_CH, 128, H).transpose(1, 0, 2)
    ).astype(f16)

    in_maps = []
    for core in range(N_CORES):
        b, h = divmod(core, 2)
        xt = xT[b]
        if h == 1:
            xt = np.concatenate([xt[:, TQ:], xt[:, :TQ]], axis=1)
        # [C, T] -> [NC_CH, NSB, 128, 512] fp16
        xt = np.ascontiguousarray(
            xt.reshape(NC_CH, 128, NSB // 2, 2, 512).transpose(0, 2, 1, 3, 4)
        ).astype(f16)
        in_maps.append({"xT": xt, "wqv": wqv, "wk": wk1, "wkv": wkv})
    return in_maps


def run(x, Wq, Wk, Wv, **spmd_kwargs):
    """Run on hardware; returns (output, BassKernelResults)."""
    nc = _get_module()
    in_maps = _make_in_maps(x, Wq, Wk, Wv)
    res = run_bass_kernel_spmd(nc, in_maps, core_ids=list(range(N_CORES)), **spmd_kwargs)
    out = np.empty((B, T, H), dtype=np.float32)
    for core in range(N_CORES):
        b, h = divmod(core, 2)
        o = np.asarray(res.results[core]["out"], dtype=np.float32)  # [65, TQ]
        out[b, h * TQ : (h + 1) * TQ, :] = (o[0:H, :] / o[H, :]).T
    return out, res


def kernel(x, Wq, Wk, Wv):
    out, _ = run(x, Wq, Wk, Wv)
    return out


# revision 15
# speedup vs baseline: 1.1869x; 1.0477x over previous
"""Single-head attention (B=4, T=4096, C=1024, H=64) on 8 trn2 NeuronCores.

Sharding: 8 shards = (batch b, query-half h).  Each core receives x[b]
pre-transposed to xT [C=1024, T=4096] in fp16; for h==1 the T columns are
rotated by 2048 so "this core's" 2048 queries are always columns 0:2048
(softmax is permutation-invariant over keys).  SPMD: identical program on
every core, no rank logic.

v2 (all-fp16 + dual-engine exp + host-side normalize):
  phase 1: stream xT in fp16 [128,512] tiles.
    blocks 0..3 (query half):  PE pass_q  [Wq*0.125 | Wv]^T x -> rows 0:64 =
    Q^T (scaled), rows 64:128 = V^T;  PE pass_k  Wk^T x -> rows 0:64 = K^T.
    blocks 4..7:               PE pass_kv [Wk | Wv]^T x  (K^T low, V^T high).
    GPSIMD copies PSUM->SBUF fp16: qt_sb[64,2048] (partitions 0:64),
    kvt_sb[128,4096] (K^T rows 0:64, V^T rows 64:128).  PE transposes V^T
    tiles (stationary from partitions 64:128) -> va [128, kt, 65] fp16, with
    a ones column at index 64 (denominator comes out of the attnV matmul).
  phase 2: per 1024-query chunk (2 of them), per key tile [128]:
    PE: scoresT [128,1024] = kvt[0:64, tile].T @ qt  (K=64 contraction, the
        1/8 softmax scale is folded into Wq host-side)
    exp, alternating by tile parity:
      ACT: ex = Exp(scoresT)                       -> fp16 SBUF
      DVE: Schraudolph bit-trick exp: ex_bits = int16(scoresT*A + B); the
        int16 bit pattern IS fp16(exp(s)) to ~1.5% rms (denominator uses the
        same approximated values, so most of the error divides out).
    PE: acc[65,512] += va[:,kt,:].T @ ex  (accumulate over 32 tiles; row 64
        of acc = sum of exp = softmax denominator)
  epilogue: GPSIMD copies acc -> fp16, DMA out [65,2048] = numerator rows
  0:64 + denominator row 64.  Host divides + transposes (the "unshard").
"""

import os
import sys

for _p in ("/opt/trn_rl_repo", "/root/.axon_site/_ro/trn_rl_repo"):
    if os.path.isdir(_p) and _p not in sys.path:
        sys.path.append(_p)

import numpy as np

import concourse.bacc as bacc
import concourse.mybir as mybir
import concourse.tile as tile
from concourse.bass_utils import run_bass_kernel_spmd
from concourse.masks import make_identity

B = 4
T = 4096
C = 1024
H = 64
TQ = T // 2  # queries per core
N_CORES = 8

F32 = mybir.dt.float32
F16 = mybir.dt.float16
I16 = mybir.dt.int16

NC_CH = C // 128  # 8 contraction chunks
NSB = T // 512  # 8 key/source blocks of 512
NST = T // 128  # 32 key tiles of 128
NTC = TQ // 512  # 4 query chunks of 512

# Schraudolph fp16 exp constants (fold the 1/8 score scale into Wq, so the
# DVE sees the already-scaled score): bits = s*1024/ln2 + (15*1024 - C).
# +0.5 turns the convert's truncation into round-half-up.  C=57 tuned by
# numpy simulation (rel-err plateau 55..65).
SCHRAUD_A = 1024.0 / float(np.log(2.0))
SCHRAUD_B = 15.0 * 1024.0 - 57.0 + 0.5


def _build_module():
    nc = bacc.Bacc("TRN2", target_bir_lowering=False, debug=False, num_devices=N_CORES)

    xT = nc.dram_tensor("xT", [NC_CH, NSB // 2, 128, 2, 512], F16, kind="ExternalInput").ap()
    # host-packed partition-major weight chunk tensors
    wqv = nc.dram_tensor("wqv", [128, NC_CH, 2 * H], F16, kind="ExternalInput").ap()
    wk = nc.dram_tensor("wk", [128, NC_CH, H], F16, kind="ExternalInput").ap()
    wkv = nc.dram_tensor("wkv", [128, NC_CH, 2 * H], F16, kind="ExternalInput").ap()
    out = nc.dram_tensor("out", [H + 1, TQ], F16, kind="ExternalOutput").ap()

    EXP = mybir.ActivationFunctionType.Exp
    MUL = mybir.AluOpType.mult
    ADD = mybir.AluOpType.add

    dma_engines = (nc.sync, nc.gpsimd, nc.scalar)

    with tile.TileContext(nc) as tc:
        with (
            tc.tile_pool(name="const", bufs=1) as const_pool,
            tc.tile_pool(name="xt", bufs=32) as xt_pool,
            tc.tile_pool(name="xt2", bufs=16) as xt2_pool,
            tc.tile_pool(name="big", bufs=1) as big_pool,
            tc.tile_pool(name="exp", bufs=10) as exp_pool,
            tc.tile_pool(name="osb", bufs=2) as out_pool,
            tc.tile_pool(name="p1", bufs=2, space="PSUM") as psum_p1,
            tc.tile_pool(name="psc", bufs=4, space="PSUM") as psum_sc,
            tc.tile_pool(name="pacc", bufs=2, space="PSUM") as psum_acc,
        ):
            # ---- constants ----
            wqv_sb = const_pool.tile([128, NC_CH, 2 * H], F16, tag="wqv")
            wk_sb = const_pool.tile([128, NC_CH, H], F16, tag="wk")
            wkv_sb = const_pool.tile([128, NC_CH, 2 * H], F16, tag="wkv")
            ident = const_pool.tile([128, 128], F16, tag="ident")
            nc.sync.dma_start(wqv_sb[:, 0:4, :], wqv[:, 0:4, :])
            nc.gpsimd.dma_start(wqv_sb[:, 4:8, :], wqv[:, 4:8, :])
            nc.scalar.dma_start(wk_sb[:, 0:4, :], wk[:, 0:4, :])
            nc.sync.dma_start(wk_sb[:, 4:8, :], wk[:, 4:8, :])
            nc.gpsimd.dma_start(wkv_sb[:], wkv)

            # ---- persistent activations ----
            kvt_sb = big_pool.tile([128, T], F16, tag="kvt")  # K^T low | V^T high
            qt_sb = big_pool.tile([64, TQ], F16, tag="qt")  # Q^T (scaled)
            va = big_pool.tile([128, NST, H + 1], F16, tag="va")

            # ---- phase 1 ----
            def emit_dma_block(sb):
                xts = []
                for c in range(NC_CH):
                    xt = xt_pool.tile([128, 512], F16, tag="xt")
                    if sb < 1:
                        # split the first block across all queues for the
                        # earliest possible first matmul
                        dma_engines[c % 3].dma_start(
                            xt[:, 0:256], xT[c, sb // 2, :, sb % 2, 0:256]
                        )
                        dma_engines[(c + 1) % 3].dma_start(
                            xt[:, 256:512], xT[c, sb // 2, :, sb % 2, 256:512]
                        )
                    else:
                        # keep the scalar sequencer free for exp work
                        dma_engines[c % 2].dma_start(xt[:], xT[c, sb // 2, :, sb % 2, :])
                    xts.append(xt)
                return xts

            kv_pair_tiles = {}

            def emit_dma_pair(sb):  # one trigger per chunk covers sb, sb+1
                xts = []
                for c in range(NC_CH):
                    xt = xt2_pool.tile([128, 2, 512], F16, tag="xt2")
                    dma_engines[c % 2].dma_start(xt[:], xT[c, sb // 2])
                    xts.append(xt)
                kv_pair_tiles[sb] = xts

            def emit_transposes(sb):
                # V^T tiles of this block -> va [128, kt, 0:64]
                for j in range(4):
                    kt = sb * 4 + j
                    vt_ps = psum_p1.tile([128, H], F16, tag="p1")
                    nc.tensor.transpose(
                        vt_ps[:],
                        kvt_sb[64:128, kt * 128 : (kt + 1) * 128],
                        ident[64:128, 64:128],
                    )
                    nc.vector.tensor_copy(va[:, kt, 0:H], vt_ps[:])

            def emit_proj_q_block(sb, xts=None):  # blocks 0..3
                if xts is None:
                    xts = emit_dma_block(sb)
                col = slice(sb * 512, (sb + 1) * 512)
                qv_ps = psum_p1.tile([128, 512], F32, tag="p1")
                for c in range(NC_CH):
                    nc.tensor.matmul(
                        qv_ps[:],
                        wqv_sb[:, c, :],
                        xts[c][:],
                        start=(c == 0),
                        stop=(c == NC_CH - 1),
                    )
                nc.vector.tensor_copy(qt_sb[:, col], qv_ps[0:64, :])
                nc.vector.tensor_copy(kvt_sb[64:128, col], qv_ps[64:128, :])
                k_ps = psum_p1.tile([64, 512], F32, tag="p1")
                for c in range(NC_CH):
                    nc.tensor.matmul(
                        k_ps[:],
                        wk_sb[:, c, :],
                        xts[c][:],
                        start=(c == 0),
                        stop=(c == NC_CH - 1),
                    )
                nc.vector.tensor_copy(kvt_sb[0:64, col], k_ps[:])
                emit_transposes(sb)

            def emit_proj_kv_block(sb):  # blocks 4..7
                base = 4 + ((sb - 4) // 2) * 2
                xts = kv_pair_tiles[base]
                half = sb - base
                col = slice(sb * 512, (sb + 1) * 512)
                kv_ps = psum_p1.tile([128, 512], F32, tag="p1")
                for c in range(NC_CH):
                    nc.tensor.matmul(
                        kv_ps[:],
                        wkv_sb[:, c, :],
                        xts[c][:, half, :],
                        start=(c == 0),
                        stop=(c == NC_CH - 1),
                    )
                nc.vector.tensor_copy(kvt_sb[:, col], kv_ps[:])
                emit_transposes(sb)

            # ---- phase 2 (software-pipelined over half-chunk tiles) ----
            # tile t = (tcp, kt, i): scores [128,512] -> exp -> attnV.
            # The PE executes its queue in order, so scores are emitted
            # LOOKAHEAD tiles ahead of the matching attnV: while the exp
            # engines work on tile t, the PE computes scores t+1..t+LA.
            acc_tiles = {}
            ex_tiles = {}
            LOOKAHEAD = 6

            def get_acc(tcp):
                if tcp not in acc_tiles:
                    if tcp == 0:
                        oa = psum_acc.tile([H + 1, 512], F32, tag="acc", name="acc_a0")
                        ob = psum_acc.tile([H + 1, 512], F32, tag="acc", name="acc_b0")
                    else:
                        # projections are done; reuse the p1 psum slots
                        oa = psum_p1.tile([H + 1, 512], F32, tag="p1", name="acc_a1")
                        ob = psum_p1.tile([H + 1, 512], F32, tag="p1", name="acc_b1")
                    acc_tiles[tcp] = (oa, ob)
                return acc_tiles[tcp]

            def emit_scores_exp(t):
                tcp, kt, i = t
                sc = psum_sc.tile([128, 512], F32, tag="sc")
                nc.tensor.matmul(
                    sc[:],
                    kvt_sb[0:64, kt * 128 : (kt + 1) * 128],
                    qt_sb[:, (2 * tcp + i) * 512 : (2 * tcp + i + 1) * 512],
                    start=True,
                    stop=True,
                )
                # ACT owns i==0, DVE owns i==1 (concurrent); in the early
                # window DVE is still doing phase-1 copies, so ACT picks up
                # some of its tiles.
                act_turn = (i == 0) ^ (kt % 2 == 1)
                if tcp == 0 and kt < 16 and kt % 3 == 2:
                    act_turn = True
                ex = exp_pool.tile([128, 512], F16, tag="ex")
                if act_turn:
                    nc.scalar.activation(ex[:], sc[:], EXP)
                else:
                    nc.vector.tensor_scalar(
                        ex[:].bitcast(I16), sc[:], SCHRAUD_A, SCHRAUD_B, MUL, ADD
                    )
                ex_tiles[t] = ex

            def emit_attnv(t):
                tcp, kt, i = t
                acc = get_acc(tcp)[i]
                nc.tensor.matmul(
                    acc[:],
                    va[:, kt, :],
                    ex_tiles.pop(t)[:],
                    start=(kt == 0),
                    stop=(kt == NST - 1),
                )

            def emit_out_half(tcp, i):
                osb = out_pool.tile([H + 1, 512], F16, tag="osb")
                nc.vector.tensor_copy(osb[:], acc_tiles[tcp][i][:])
                nc.sync.dma_start(
                    out[:, tcp * 1024 + i * 512 : tcp * 1024 + (i + 1) * 512],
                    osb[:],
                )

            xts0 = emit_dma_block(0)
            warm_sb = big_pool.tile([128, 512], F16, tag="warm")
            nc.gpsimd.memset(warm_sb[:], 0.0)
            for _ in range(10):
                warm_ps = psum_sc.tile([128, 512], F32, tag="sc")
                nc.tensor.matmul(
                    warm_ps[:], warm_sb[:, 0:128], warm_sb[:], start=True, stop=True
                )
            make_identity(nc, ident[:])
            nc.gpsimd.memset(va[:, :, H : H + 1], 1.0)
            emit_proj_q_block(0, xts0)
            for sb in range(1, NTC):
                emit_proj_q_block(sb)
            emit_dma_pair(4)
            emit_dma_pair(6)

            tiles = [(tcp, kt, i) for tcp in (0, 1) for kt in range(NST)
                     for i in (0, 1)]
            for idx in range(len(tiles) + LOOKAHEAD):
                if idx < len(tiles):
                    tcp, kt, i = tiles[idx]
                    # stream the key-half projection blocks through the
                    # early attention tiles
                    if tcp == 0 and i == 0 and kt < 16 and kt % 4 == 0:
                        emit_proj_kv_block(NTC + kt // 4)
                    emit_scores_exp(tiles[idx])
                if idx >= LOOKAHEAD:
                    t = tiles[idx - LOOKAHEAD]
                    emit_attnv(t)
                    if t[1] == NST - 1:
                        emit_out_half(t[0], t[2])

    nc.compile()
    return nc


_NC_CACHE = None


def _get_module():
    global _NC_CACHE
    if _NC_CACHE is None:
        _NC_CACHE = _build_module()
    return _NC_CACHE


def _make_in_maps(x, Wq, Wk, Wv):
    f16 = np.float16
    xT = np.transpose(np.asarray(x, dtype=np.float32), (0, 2, 1))  # [B, C, T]
    wq = np.asarray(Wq, dtype=np.float32) * 0.125  # fold softmax scale
    wk_ = np.asarray(Wk, dtype=np.float32)
    wv = np.asarray(Wv, dtype=np.float32)

    def pack2(a, b):  # [C, H] x2 -> [128, NC_CH, 2H] partition-major fp16
        cat = np.concatenate([a, b], axis=1).reshape(NC_CH, 128, 2 * H)
        return np.ascontiguousarray(cat.transpose(1, 0, 2)).astype(f16)

    wqv = pack2(wq, wv)
    wkv = pack2(wk_, wv)
    wk1 = np.ascontiguousarray(
        wk_.reshape(NC_CH, 128, H).transpose(1, 0, 2)
    ).astype(f16)

    in_maps = []
    for core in range(N_CORES):
        b, h = divmod(core, 2)
        xt = xT[b]
        if h == 1:
            xt = np.concatenate([xt[:, TQ:], xt[:, :TQ]], axis=1)
        # [C, T] -> [NC_CH, NSB, 128, 512] fp16
        xt = np.ascontiguousarray(
            xt.reshape(NC_CH, 128, NSB // 2, 2, 512).transpose(0, 2, 1, 3, 4)
        ).astype(f16)
        in_maps.append({"xT": xt, "wqv": wqv, "wk": wk1, "wkv": wkv})
    return in_maps


def run(x, Wq, Wk, Wv, **spmd_kwargs):
    """Run on hardware; returns (output, BassKernelResults)."""
    nc = _get_module()
    in_maps = _make_in_maps(x, Wq, Wk, Wv)
    res = run_bass_kernel_spmd(nc, in_maps, core_ids=list(range(N_CORES)), **spmd_kwargs)
    out = np.empty((B, T, H), dtype=np.float32)
    for core in range(N_CORES):
        b, h = divmod(core, 2)
        o = np.asarray(res.results[core]["out"], dtype=np.float32)  # [65, TQ]
        out[b, h * TQ : (h + 1) * TQ, :] = (o[0:H, :] / o[H, :]).T
    return out, res


def kernel(x, Wq, Wk, Wv):
    out, _ = run(x, Wq, Wk, Wv)
    return out
